# revision 6
# baseline (speedup 1.0000x reference)
"""Trainium2 Bass kernel for Bahdanau-style attention (nn_BahdanauAttention).

Reference computation (per layer l, case c):
    q[l,c,:]   = queries[l,0,c,:] @ Qw[l].T + Qb[l]            # [H]
    k[l,t,c,:] = keys[l,t,c,:] @ Kw[l].T + Kb[l]               # [H]
    s[l,t,c]   = Vw[l] . tanh(q + k) + Vb[l]
    w          = softmax over flattened (l,t) per case c
    context[c] = sum_{l,t} w[l,t,c] * keys[l,t,c,:]            # [KL]

Sharding: cases (C=32) split across 8 NeuronCores, 4 cases per core.
All reductions are per-case, so cores are fully independent.

Per-core dataflow (all heavy compute in bf16, accumulation in fp32):
  - keys streamed HBM->SBUF once per (l, case) with fp32->bf16 cast in the
    DMA (SWDGE). Tiles stay resident for the whole case (8 MiB/case).
  - PE transposes 128x128 blocks to get kl-on-partitions for the projection
    matmul; DVE evacuates PSUM->SBUF.
  - Projection k^T = KwT.T @ keysT on PE (bf16), tanh(+bias q+Qb+Kb) on ACT,
    score reduction over H on PE, producing scores with t on partitions.
  - Global softmax per case on DVE/ACT (+ PE for cross-partition reductions).
  - context = e^T-weighted sum of keys on PE, accumulated in PSUM, scaled by
    1/S at the end.
"""

import os
import sys

import numpy as np

sys.path.insert(0, "/opt/trn_rl_repo")

import concourse.bacc as bacc
import concourse.bass as bass
import concourse.mybir as mybir
import concourse.tile as tile
from concourse import bass_utils
from concourse.masks import make_identity

L, T, C, QL, KL, H = 4, 2048, 32, 512, 512, 256
NCORES = 8
CL = C // NCORES  # local cases per core
P = 128
TC = T // P       # 16 t-chunks of 128 per (l, case)
KC = KL // P      # 4 kl chunks
QC = QL // P      # 4 ql chunks
HC = H // P       # 2 h chunks
TS = 512          # t-slab width for projection matmuls
NTS = T // TS     # 4 slabs per (l, case)
TB = TS // P      # 4 t-blocks of 128 per slab

F32 = mybir.dt.float32
BF16 = mybir.dt.bfloat16
AX = mybir.AxisListType
AF = mybir.ActivationFunctionType


def build_kernel_body(tc):
    from contextlib import ExitStack
    with ExitStack() as ctx:
        _build_kernel_body(tc, ctx)


def _build_kernel_body(tc, ctx):
    nc = tc.nc
    keys_d = nc.dram_tensor("keys", [L, T, CL, KL], F32, kind="ExternalInput").ap()
    qT_d = nc.dram_tensor("queriesT", [L, QL, CL], F32, kind="ExternalInput").ap()
    kwT_d = nc.dram_tensor("KwT", [L, KL, H], F32, kind="ExternalInput").ap()
    qwT_d = nc.dram_tensor("QwT", [L, QL, H], F32, kind="ExternalInput").ap()
    qbR_d = nc.dram_tensor("QbR", [L, P, HC], F32, kind="ExternalInput").ap()
    kbR_d = nc.dram_tensor("KbR", [L, P, HC], F32, kind="ExternalInput").ap()
    vwR_d = nc.dram_tensor("VwR", [P, HC, L], F32, kind="ExternalInput").ap()
    vb_d = nc.dram_tensor("Vb", [1, L], F32, kind="ExternalInput").ap()
    wout_d = nc.dram_tensor("wout", [CL, P, L, TC], F32, kind="ExternalOutput").ap()
    ctx_d = nc.dram_tensor("ctx", [CL, KL], F32, kind="ExternalOutput").ap()

    const = ctx.enter_context(tc.tile_pool(name="const", bufs=1))
    knat_pool = ctx.enter_context(tc.tile_pool(name="knat", bufs=2 * L))
    ktr_pool = ctx.enter_context(tc.tile_pool(name="ktr", bufs=3))
    h_pool = ctx.enter_context(tc.tile_pool(name="hpool", bufs=3))
    sc_pool = ctx.enter_context(tc.tile_pool(name="scpool", bufs=2))
    sm_pool = ctx.enter_context(tc.tile_pool(name="smpool", bufs=2))
    tr_psum = ctx.enter_context(tc.tile_pool(name="trps", bufs=2, space="PSUM"))
    kt_psum = ctx.enter_context(tc.tile_pool(name="ktps", bufs=3, space="PSUM"))
    sp_psum = ctx.enter_context(tc.tile_pool(name="spps", bufs=2, space="PSUM"))
    ctx_psum = ctx.enter_context(tc.tile_pool(name="ctxps", bufs=1, space="PSUM"))

    # ---- constants / weights --------------------------------------------
    ident_bf = const.tile([P, P], BF16)
    make_identity(nc, ident_bf)
    ident_f32 = const.tile([P, P], F32)
    make_identity(nc, ident_f32)
    ones_col = const.tile([P, 1], F32)
    nc.vector.memset(ones_col, 1.0)
    ones_row = const.tile([1, P], F32)
    nc.vector.memset(ones_row, 1.0)

    kwT_sb = const.tile([P, L, KC, H], BF16)
    nc.gpsimd.dma_start(out=kwT_sb, in_=kwT_d.rearrange("l (kc p) h -> p l kc h", p=P))
    qwT_sb = const.tile([P, L, QC, H], BF16)
    nc.gpsimd.dma_start(out=qwT_sb, in_=qwT_d.rearrange("l (qc p) h -> p l qc h", p=P))
    qq_sb = const.tile([P, L, QC, CL], BF16)
    nc.gpsimd.dma_start(out=qq_sb, in_=qT_d.rearrange("l (qc p) c -> p l qc c", p=P))
    vw_sb = const.tile([P, HC, L], BF16)
    nc.gpsimd.dma_start(out=vw_sb, in_=vwR_d)
    vb_sb = const.tile([1, L], F32)
    nc.gpsimd.dma_start(out=vb_sb, in_=vb_d)
    vb_bc = const.tile([P, L], F32)
    nc.gpsimd.dma_start(out=vb_bc, in_=vb_d.to_broadcast((P, L)))
    qb_sb = const.tile([P, L, HC, 1], F32)
    nc.gpsimd.dma_start(out=qb_sb, in_=qbR_d.rearrange("l p h -> p l h"))
    kb_sb = const.tile([P, L, HC, 1], F32)
    nc.gpsimd.dma_start(out=kb_sb, in_=kbR_d.rearrange("l p h -> p l h"))
    qbkb = const.tile([P, L, HC, 1], F32)
    nc.vector.tensor_add(out=qbkb, in0=qb_sb, in1=kb_sb)

    # ---- q projection: bias_all[p_h, l, hc, c] = q + Qb + Kb ------------
    bias_all = const.tile([P, L, HC, CL], F32)
    for l in range(L):
        qps = sp_psum.tile([P, HC, CL], F32, tag="small", name=f"qps{l}")
        for hc in range(HC):
            for qc in range(QC):
                nc.tensor.matmul(
                    out=qps[:, hc, :],
                    lhsT=qwT_sb[:, l, qc, hc * P:(hc + 1) * P],
                    rhs=qq_sb[:, l, qc, :],
                    start=(qc == 0),
                    stop=(qc == QC - 1),
                )
        nc.vector.tensor_add(
            out=bias_all[:, l],
            in0=qps,
            in1=qbkb[:, l].to_broadcast((P, HC, CL)),
        )

    # ---- main loop over cases -------------------------------------------
    for ci in range(CL):
        knat_tiles = []
        scores_sb = sc_pool.tile([P, L, TC], F32, tag="scores", name=f"scores_c{ci}")
        for l in range(L):
            knat = knat_pool.tile([P, TC, KL], BF16, tag="knat", name=f"knat_c{ci}l{l}")
            knat_tiles.append(knat)
            nc.gpsimd.dma_start(
                out=knat, in_=keys_d[l, :, ci, :].rearrange("(t p) k -> p t k", p=P)
            )
            scps = sp_psum.tile([P, TC], F32, tag="small", name=f"scps_c{ci}l{l}")
            for ts in range(NTS):
                # transpose the slab: ktr[p_kl, kc, j] = keys[t=ts*TS+j, kl=kc*P+p_kl]
                ktr = ktr_pool.tile([P, KC, TS], BF16, tag="ktr", name=f"ktr{ci}_{l}_{ts}")
                for kc in range(KC):
                    tps = tr_psum.tile([P, TS], BF16, tag="tps", name=f"tps{ci}_{l}_{ts}_{kc}")
                    for tb in range(TB):
                        nc.tensor.transpose(
                            out=tps[:, tb * P:(tb + 1) * P],
                            in_=knat[:, ts * TB + tb, kc * P:(kc + 1) * P],
                            identity=ident_bf,
                        )
                    nc.vector.tensor_copy(out=ktr[:, kc, :], in_=tps)
                # projection + tanh: hT[p_h, hc, j] = tanh(q + k + biases)
                hT = h_pool.tile([P, HC, TS], BF16, tag="hT", name=f"hT{ci}_{l}_{ts}")
                for hc in range(HC):
                    kt = kt_psum.tile([P, TS], F32, tag="ktps", name=f"kt{ci}_{l}_{ts}_{hc}")
                    for kc in range(KC):
                        nc.tensor.matmul(
                            out=kt,
                            lhsT=kwT_sb[:, l, kc, hc * P:(hc + 1) * P],
                            rhs=ktr[:, kc, :],
                            start=(kc == 0),
                            stop=(kc == KC - 1),
                        )
                    nc.scalar.activation(
                        out=hT[:, hc, :],
                        in_=kt,
                        func=AF.Tanh,
                        bias=bias_all[:, l, hc, ci:ci + 1],
                        scale=1.0,
                    )
                # scores: scps[p_t, ts*TB+tb] = sum_h hT * Vw
                for tb in range(TB):
                    col = ts * TB + tb
                    for hc in range(HC):
                        nc.tensor.matmul(
                            out=scps[:, col:col + 1],
                            lhsT=hT[:, hc, tb * P:(tb + 1) * P],
                            rhs=vw_sb[:, hc, l:l + 1],
                            start=(hc == 0),
                            stop=(hc == HC - 1),
                        )
            nc.vector.tensor_copy(out=scores_sb[:, l, :], in_=scps)

        # ---- softmax over all (l, t) for this case ----------------------
        mx4 = sm_pool.tile([P, L], F32, tag="mx4", name=f"mx4_{ci}")
        nc.vector.reduce_max(out=mx4, in_=scores_sb, axis=AX.X)
        nc.vector.tensor_add(out=mx4, in0=mx4, in1=vb_bc)
        mcol = sm_pool.tile([P, 1], F32, tag="mcol", name=f"mcol_{ci}")
        nc.vector.reduce_max(out=mcol, in_=mx4, axis=AX.X)
        mrow_ps = sp_psum.tile([1, P], F32, tag="small", name=f"mrow_{ci}")
        nc.tensor.transpose(out=mrow_ps, in_=mcol, identity=ident_f32)
        m1 = sm_pool.tile([1, 1], F32, tag="m1", name=f"m1_{ci}")
        nc.vector.reduce_max(out=m1, in_=mrow_ps, axis=AX.X)
        # exp bias per l: Vb[l] - m
        vbm = sm_pool.tile([1, L], F32, tag="vbm", name=f"vbm_{ci}")
        nc.vector.tensor_scalar(
            out=vbm, in0=vb_sb, scalar1=m1, scalar2=None,
            op0=mybir.AluOpType.subtract,
        )
        ebias_ps = sp_psum.tile([P, L], F32, tag="small", name=f"ebias_ps_{ci}")
        nc.tensor.matmul(out=ebias_ps, lhsT=ones_row, rhs=vbm, start=True, stop=True)
        ebias = sm_pool.tile([P, L], F32, tag="ebias", name=f"ebias_{ci}")
        nc.vector.tensor_copy(out=ebias, in_=ebias_ps)
        e_sb = sm_pool.tile([P, L, TC], F32, tag="esb", name=f"esb_{ci}")
        esums = sm_pool.tile([P, L], F32, tag="esums", name=f"esums_{ci}")
        for l in range(L):
            nc.scalar.activation(
                out=e_sb[:, l, :],
                in_=scores_sb[:, l, :],
                func=AF.Exp,
                bias=ebias[:, l:l + 1],
                scale=1.0,
                accum_out=esums[:, l:l + 1],
            )
        scol = sm_pool.tile([P, 1], F32, tag="scol", name=f"scol_{ci}")
        nc.vector.reduce_sum(out=scol, in_=esums, axis=AX.X)
        sps = sp_psum.tile([1, 1], F32, tag="small", name=f"sps_{ci}")
        nc.tensor.matmul(out=sps, lhsT=scol, rhs=ones_col, start=True, stop=True)
        rs = sm_pool.tile([1, 1], F32, tag="rs", name=f"rs_{ci}")
        nc.vector.reciprocal(out=rs, in_=sps)
        rs_ps = sp_psum.tile([P, 1], F32, tag="small", name=f"rs_ps_{ci}")
        nc.tensor.matmul(out=rs_ps, lhsT=ones_row, rhs=rs, start=True, stop=True)
        rs_bc = sm_pool.tile([P, 1], F32, tag="rsbc", name=f"rsbc_{ci}")
        nc.vector.tensor_copy(out=rs_bc, in_=rs_ps)
        # normalized weights out
        wnorm = sm_pool.tile([P, L, TC], F32, tag="wnorm", name=f"wnorm_{ci}")
        nc.vector.tensor_scalar_mul(out=wnorm, in0=e_sb, scalar1=rs_bc)
        nc.sync.dma_start(out=wout_d[ci], in_=wnorm)
        # context
        e_bf = sm_pool.tile([P, L, TC], BF16, tag="ebf", name=f"ebf_{ci}")
        nc.vector.tensor_copy(out=e_bf, in_=e_sb)
        cps = ctx_psum.tile([1, KL], F32, tag="ctx", name=f"cps_{ci}")
        n = 0
        for l in range(L):
            for tcb in range(TC):
                nc.tensor.matmul(
                    out=cps,
                    lhsT=e_bf[:, l, tcb:tcb + 1],
                    rhs=knat_tiles[l][:, tcb, :],
                    start=(n == 0),
                    stop=(n == L * TC - 1),
                )
                n += 1
        ctx_sb = sm_pool.tile([1, KL], F32, tag="ctxsb", name=f"ctxsb_{ci}")
        nc.vector.tensor_scalar_mul(out=ctx_sb, in0=cps, scalar1=rs)
        nc.sync.dma_start(out=ctx_d[ci:ci + 1, :], in_=ctx_sb)


_BUILT = None


def build():
    global _BUILT
    if _BUILT is None:
        nc = bacc.Bacc(
            "TRN2",
            target_bir_lowering=False,
            debug=False,
            enable_asserts=False,
            num_devices=NCORES,
        )
        with tile.TileContext(nc) as tc:
            build_kernel_body(tc)
        nc.compile()
        _BUILT = nc
    return _BUILT


def make_in_maps(queries, keys, Qw, Qb, Kw, Kb, Vw, Vb):
    """Host-side sharding + pure layout prep (transposes/reshapes only)."""
    queries = np.ascontiguousarray(np.asarray(queries, dtype=np.float32))
    keys = np.asarray(keys, dtype=np.float32)
    KwT = np.ascontiguousarray(np.asarray(Kw, np.float32).transpose(0, 2, 1))  # [L,KL,H]
    QwT = np.ascontiguousarray(np.asarray(Qw, np.float32).transpose(0, 2, 1))  # [L,QL,H]
    QbR = np.ascontiguousarray(
        np.asarray(Qb, np.float32).reshape(L, HC, P).transpose(0, 2, 1))  # [L,P,HC]
    KbR = np.ascontiguousarray(
        np.asarray(Kb, np.float32).reshape(L, HC, P).transpose(0, 2, 1))  # [L,P,HC]
    VwR = np.ascontiguousarray(
        np.asarray(Vw, np.float32).T.reshape(HC, P, L).transpose(1, 0, 2))  # [P,HC,L]
    Vb2 = np.asarray(Vb, np.float32).reshape(1, L)
    in_maps = []
    for core in range(NCORES):
        cs = slice(core * CL, (core + 1) * CL)
        in_maps.append({
            "keys": np.ascontiguousarray(keys[:, :, cs, :]),
            "queriesT": np.ascontiguousarray(queries[:, 0, cs, :].transpose(0, 2, 1)),
            "KwT": KwT, "QwT": QwT, "QbR": QbR, "KbR": KbR, "VwR": VwR, "Vb": Vb2,
        })
    return in_maps


def assemble_outputs(results):
    ctx = np.concatenate([r["ctx"] for r in results], axis=0)  # [C, KL]
    wparts = []
    for r in results:
        w = r["wout"]  # [CL, P, L, TC];  w[c, p, l, tc] = weight[l, t=tc*P+p, c]
        wparts.append(np.ascontiguousarray(w.transpose(2, 3, 1, 0)).reshape(L, T, CL))
    weights = np.concatenate(wparts, axis=2).reshape(L, T, C, 1)
    return ctx.astype(np.float32), weights.astype(np.float32)


def kernel(queries, keys, Qw, Qb, Kw, Kb, Vw, Vb, _trace=False, _tmpdir=None):
    nc = build()
    in_maps = make_in_maps(queries, keys, Qw, Qb, Kw, Kb, Vw, Vb)
    res = bass_utils.run_bass_kernel_spmd(
        nc, in_maps, core_ids=list(range(NCORES)), trace=_trace, tmpdir=_tmpdir,
    )
    out = assemble_outputs(res.results)
    if _trace:
        kernel.last_results = res
    return out


# revision 8
# speedup vs baseline: 1.2806x; 1.2806x over previous
"""Trainium2 Bass kernel for Bahdanau-style attention (nn_BahdanauAttention).

Reference computation (per layer l, case c):
    q[l,c,:]   = queries[l,0,c,:] @ Qw[l].T + Qb[l]            # [H]
    k[l,t,c,:] = keys[l,t,c,:] @ Kw[l].T + Kb[l]               # [H]
    s[l,t,c]   = Vw[l] . tanh(q + k) + Vb[l]
    w          = softmax over flattened (l,t) per case c
    context[c] = sum_{l,t} w[l,t,c] * keys[l,t,c,:]            # [KL]

Sharding: cases (C=32) split across 8 NeuronCores, 4 cases per core.
All reductions are per-case, so cores are fully independent.

Per-core dataflow (all heavy compute in bf16, accumulation in fp32):
  - keys streamed HBM->SBUF once per (l, case) with fp32->bf16 cast in the
    DMA (SWDGE). Tiles stay resident for the whole case (8 MiB/case).
  - PE transposes 128x128 blocks to get kl-on-partitions for the projection
    matmul; DVE evacuates PSUM->SBUF.
  - Projection k^T = KwT.T @ keysT on PE (bf16), tanh(+bias q+Qb+Kb) on ACT,
    score reduction over H on PE, producing scores with t on partitions.
  - Global softmax per case on DVE/ACT (+ PE for cross-partition reductions).
  - context = e^T-weighted sum of keys on PE, accumulated in PSUM, scaled by
    1/S at the end.
"""

import os
import sys

import numpy as np

sys.path.insert(0, "/opt/trn_rl_repo")

import concourse.bacc as bacc
import concourse.bass as bass
import concourse.mybir as mybir
import concourse.tile as tile
from concourse import bass_utils
from concourse.masks import make_identity

L, T, C, QL, KL, H = 4, 2048, 32, 512, 512, 256
NCORES = 8
CL = C // NCORES  # local cases per core
P = 128
TC = T // P       # 16 t-chunks of 128 per (l, case)
KC = KL // P      # 4 kl chunks
QC = QL // P      # 4 ql chunks
HC = H // P       # 2 h chunks
TS = 512          # t-slab width for projection matmuls
NTS = T // TS     # 4 slabs per (l, case)
TB = TS // P      # 4 t-blocks of 128 per slab

F32 = mybir.dt.float32
BF16 = mybir.dt.bfloat16
AX = mybir.AxisListType
AF = mybir.ActivationFunctionType


def build_kernel_body(tc):
    from contextlib import ExitStack
    with ExitStack() as ctx:
        _build_kernel_body(tc, ctx)


def _build_kernel_body(tc, ctx):
    nc = tc.nc
    keys_d = nc.dram_tensor("keys", [L, T, CL, KL], F32, kind="ExternalInput").ap()
    qT_d = nc.dram_tensor("queriesT", [L, QL, CL], F32, kind="ExternalInput").ap()
    kwT_d = nc.dram_tensor("KwT", [L, KL, H], F32, kind="ExternalInput").ap()
    qwT_d = nc.dram_tensor("QwT", [L, QL, H], F32, kind="ExternalInput").ap()
    qbR_d = nc.dram_tensor("QbR", [L, P, HC], F32, kind="ExternalInput").ap()
    kbR_d = nc.dram_tensor("KbR", [L, P, HC], F32, kind="ExternalInput").ap()
    vwR_d = nc.dram_tensor("VwR", [P, HC, L], F32, kind="ExternalInput").ap()
    vb_d = nc.dram_tensor("Vb", [1, L], F32, kind="ExternalInput").ap()
    wout_d = nc.dram_tensor("wout", [CL, P, L, TC], F32, kind="ExternalOutput").ap()
    ctx_d = nc.dram_tensor("ctx", [CL, KL], F32, kind="ExternalOutput").ap()

    const = ctx.enter_context(tc.tile_pool(name="const", bufs=1))
    knat_pool = ctx.enter_context(tc.tile_pool(name="knat", bufs=2 * L))
    ktr_pool = ctx.enter_context(tc.tile_pool(name="ktr", bufs=4))
    h_pool = ctx.enter_context(tc.tile_pool(name="hpool", bufs=4))
    sc_pool = ctx.enter_context(tc.tile_pool(name="scpool", bufs=2))
    sm_pool = ctx.enter_context(tc.tile_pool(name="smpool", bufs=2))
    tr_psum = ctx.enter_context(tc.tile_pool(name="trps", bufs=3, space="PSUM"))
    kt_psum = ctx.enter_context(tc.tile_pool(name="ktps", bufs=2, space="PSUM"))
    sp_psum = ctx.enter_context(tc.tile_pool(name="spps", bufs=2, space="PSUM"))
    ctx_psum = ctx.enter_context(tc.tile_pool(name="ctxps", bufs=1, space="PSUM"))

    # ---- constants / weights --------------------------------------------
    ident_bf = const.tile([P, P], BF16)
    make_identity(nc, ident_bf)
    ident_f32 = const.tile([P, P], F32)
    make_identity(nc, ident_f32)
    ones_col = const.tile([P, 1], F32)
    nc.vector.memset(ones_col, 1.0)
    ones_row = const.tile([1, P], F32)
    nc.vector.memset(ones_row, 1.0)

    kwT_sb = const.tile([P, L, KC, H], BF16)
    nc.gpsimd.dma_start(out=kwT_sb, in_=kwT_d.rearrange("l (kc p) h -> p l kc h", p=P))
    qwT_sb = const.tile([P, L, QC, H], BF16)
    nc.gpsimd.dma_start(out=qwT_sb, in_=qwT_d.rearrange("l (qc p) h -> p l qc h", p=P))
    qq_sb = const.tile([P, L, QC, CL], BF16)
    nc.gpsimd.dma_start(out=qq_sb, in_=qT_d.rearrange("l (qc p) c -> p l qc c", p=P))
    vw_sb = const.tile([P, HC, L], BF16)
    nc.gpsimd.dma_start(out=vw_sb, in_=vwR_d)
    vb_sb = const.tile([1, L], F32)
    nc.gpsimd.dma_start(out=vb_sb, in_=vb_d)
    vb_bc = const.tile([P, L], F32)
    nc.gpsimd.dma_start(out=vb_bc, in_=vb_d.to_broadcast((P, L)))
    qb_sb = const.tile([P, L, HC, 1], F32)
    nc.gpsimd.dma_start(out=qb_sb, in_=qbR_d.rearrange("l p h -> p l h"))
    kb_sb = const.tile([P, L, HC, 1], F32)
    nc.gpsimd.dma_start(out=kb_sb, in_=kbR_d.rearrange("l p h -> p l h"))
    qbkb = const.tile([P, L, HC, 1], F32)
    nc.vector.tensor_add(out=qbkb, in0=qb_sb, in1=kb_sb)

    # ---- q projection: bias_all[p_h, l, hc, c] = q + Qb + Kb ------------
    bias_all = const.tile([P, L, HC, CL], F32)
    for l in range(L):
        qps = sp_psum.tile([P, HC, CL], F32, tag="small", name=f"qps{l}")
        for hc in range(HC):
            for qc in range(QC):
                nc.tensor.matmul(
                    out=qps[:, hc, :],
                    lhsT=qwT_sb[:, l, qc, hc * P:(hc + 1) * P],
                    rhs=qq_sb[:, l, qc, :],
                    start=(qc == 0),
                    stop=(qc == QC - 1),
                )
        nc.vector.tensor_add(
            out=bias_all[:, l],
            in0=qps,
            in1=qbkb[:, l].to_broadcast((P, HC, CL)),
        )

    # ---- main loop over cases -------------------------------------------
    for ci in range(CL):
        knat_tiles = []
        scores_sb = sc_pool.tile([P, L, TC], F32, tag="scores", name=f"scores_c{ci}")
        for l in range(L):
            knat = knat_pool.tile([P, TC, KL], BF16, tag="knat", name=f"knat_c{ci}l{l}")
            knat_tiles.append(knat)
            keys_r = keys_d[l, :, ci, :].rearrange("(t p) k -> p t k", p=P)
            for ts in range(NTS):
                sl = slice(ts * TB, (ts + 1) * TB)
                nc.gpsimd.dma_start(out=knat[:, sl, :], in_=keys_r[:, sl, :])
            scps = sp_psum.tile([P, TC], F32, tag="small", name=f"scps_c{ci}l{l}")
            for ts in range(NTS):
                # transpose the slab: ktr[p_kl, kc, j] = keys[t=ts*TS+j, kl=kc*P+p_kl]
                ktr = ktr_pool.tile([P, KC, TS], BF16, tag="ktr", name=f"ktr{ci}_{l}_{ts}")
                for kc in range(KC):
                    tps = tr_psum.tile([P, TS], BF16, tag="tps", name=f"tps{ci}_{l}_{ts}_{kc}")
                    for tb in range(TB):
                        nc.tensor.transpose(
                            out=tps[:, tb * P:(tb + 1) * P],
                            in_=knat[:, ts * TB + tb, kc * P:(kc + 1) * P],
                            identity=ident_bf,
                        )
                    nc.vector.tensor_copy(out=ktr[:, kc, :], in_=tps)
                # projection + tanh: hT[p_h, hc, j] = tanh(q + k + biases)
                hT = h_pool.tile([P, HC, TS], BF16, tag="hT", name=f"hT{ci}_{l}_{ts}")
                for hc in range(HC):
                    kt = kt_psum.tile([P, TS], F32, tag="ktps", name=f"kt{ci}_{l}_{ts}_{hc}")
                    for kc in range(KC):
                        nc.tensor.matmul(
                            out=kt,
                            lhsT=kwT_sb[:, l, kc, hc * P:(hc + 1) * P],
                            rhs=ktr[:, kc, :],
                            start=(kc == 0),
                            stop=(kc == KC - 1),
                        )
                    nc.scalar.activation(
                        out=hT[:, hc, :],
                        in_=kt,
                        func=AF.Tanh,
                        bias=bias_all[:, l, hc, ci:ci + 1],
                        scale=1.0,
                    )
                # scores: scps[p_t, ts*TB+tb] = sum_h hT * Vw
                for tb in range(TB):
                    col = ts * TB + tb
                    for hc in range(HC):
                        nc.tensor.matmul(
                            out=scps[:, col:col + 1],
                            lhsT=hT[:, hc, tb * P:(tb + 1) * P],
                            rhs=vw_sb[:, hc, l:l + 1],
                            start=(hc == 0),
                            stop=(hc == HC - 1),
                        )
            nc.vector.tensor_copy(out=scores_sb[:, l, :], in_=scps)

        # ---- softmax over all (l, t) for this case ----------------------
        mx4 = sm_pool.tile([P, L], F32, tag="mx4", name=f"mx4_{ci}")
        nc.vector.reduce_max(out=mx4, in_=scores_sb, axis=AX.X)
        nc.vector.tensor_add(out=mx4, in0=mx4, in1=vb_bc)
        mcol = sm_pool.tile([P, 1], F32, tag="mcol", name=f"mcol_{ci}")
        nc.vector.reduce_max(out=mcol, in_=mx4, axis=AX.X)
        mrow_ps = sp_psum.tile([1, P], F32, tag="small", name=f"mrow_{ci}")
        nc.tensor.transpose(out=mrow_ps, in_=mcol, identity=ident_f32)
        m1 = sm_pool.tile([1, 1], F32, tag="m1", name=f"m1_{ci}")
        nc.vector.reduce_max(out=m1, in_=mrow_ps, axis=AX.X)
        # exp bias per l: Vb[l] - m
        vbm = sm_pool.tile([1, L], F32, tag="vbm", name=f"vbm_{ci}")
        nc.vector.tensor_scalar(
            out=vbm, in0=vb_sb, scalar1=m1, scalar2=None,
            op0=mybir.AluOpType.subtract,
        )
        ebias_ps = sp_psum.tile([P, L], F32, tag="small", name=f"ebias_ps_{ci}")
        nc.tensor.matmul(out=ebias_ps, lhsT=ones_row, rhs=vbm, start=True, stop=True)
        ebias = sm_pool.tile([P, L], F32, tag="ebias", name=f"ebias_{ci}")
        nc.vector.tensor_copy(out=ebias, in_=ebias_ps)
        e_sb = sm_pool.tile([P, L, TC], F32, tag="esb", name=f"esb_{ci}")
        esums = sm_pool.tile([P, L], F32, tag="esums", name=f"esums_{ci}")
        for l in range(L):
            nc.scalar.activation(
                out=e_sb[:, l, :],
                in_=scores_sb[:, l, :],
                func=AF.Exp,
                bias=ebias[:, l:l + 1],
                scale=1.0,
                accum_out=esums[:, l:l + 1],
            )
        scol = sm_pool.tile([P, 1], F32, tag="scol", name=f"scol_{ci}")
        nc.vector.reduce_sum(out=scol, in_=esums, axis=AX.X)
        sps = sp_psum.tile([1, 1], F32, tag="small", name=f"sps_{ci}")
        nc.tensor.matmul(out=sps, lhsT=scol, rhs=ones_col, start=True, stop=True)
        rs = sm_pool.tile([1, 1], F32, tag="rs", name=f"rs_{ci}")
        nc.vector.reciprocal(out=rs, in_=sps)
        rs_ps = sp_psum.tile([P, 1], F32, tag="small", name=f"rs_ps_{ci}")
        nc.tensor.matmul(out=rs_ps, lhsT=ones_row, rhs=rs, start=True, stop=True)
        rs_bc = sm_pool.tile([P, 1], F32, tag="rsbc", name=f"rsbc_{ci}")
        nc.vector.tensor_copy(out=rs_bc, in_=rs_ps)
        # normalized weights out
        wnorm = sm_pool.tile([P, L, TC], F32, tag="wnorm", name=f"wnorm_{ci}")
        nc.vector.tensor_scalar_mul(out=wnorm, in0=e_sb, scalar1=rs_bc)
        nc.sync.dma_start(out=wout_d[ci], in_=wnorm)
        # context
        e_bf = sm_pool.tile([P, L, TC], BF16, tag="ebf", name=f"ebf_{ci}")
        nc.vector.tensor_copy(out=e_bf, in_=e_sb)
        cps = ctx_psum.tile([1, KL], F32, tag="ctx", name=f"cps_{ci}")
        n = 0
        for l in range(L):
            for tcb in range(TC):
                nc.tensor.matmul(
                    out=cps,
                    lhsT=e_bf[:, l, tcb:tcb + 1],
                    rhs=knat_tiles[l][:, tcb, :],
                    start=(n == 0),
                    stop=(n == L * TC - 1),
                )
                n += 1
        ctx_sb = sm_pool.tile([1, KL], F32, tag="ctxsb", name=f"ctxsb_{ci}")
        nc.vector.tensor_scalar_mul(out=ctx_sb, in0=cps, scalar1=rs)
        nc.sync.dma_start(out=ctx_d[ci:ci + 1, :], in_=ctx_sb)


_BUILT = None


def build():
    global _BUILT
    if _BUILT is None:
        nc = bacc.Bacc(
            "TRN2",
            target_bir_lowering=False,
            debug=False,
            enable_asserts=False,
            num_devices=NCORES,
        )
        with tile.TileContext(nc) as tc:
            build_kernel_body(tc)
        nc.compile()
        _BUILT = nc
    return _BUILT


def make_in_maps(queries, keys, Qw, Qb, Kw, Kb, Vw, Vb):
    """Host-side sharding + pure layout prep (transposes/reshapes only)."""
    queries = np.ascontiguousarray(np.asarray(queries, dtype=np.float32))
    keys = np.asarray(keys, dtype=np.float32)
    KwT = np.ascontiguousarray(np.asarray(Kw, np.float32).transpose(0, 2, 1))  # [L,KL,H]
    QwT = np.ascontiguousarray(np.asarray(Qw, np.float32).transpose(0, 2, 1))  # [L,QL,H]
    QbR = np.ascontiguousarray(
        np.asarray(Qb, np.float32).reshape(L, HC, P).transpose(0, 2, 1))  # [L,P,HC]
    KbR = np.ascontiguousarray(
        np.asarray(Kb, np.float32).reshape(L, HC, P).transpose(0, 2, 1))  # [L,P,HC]
    VwR = np.ascontiguousarray(
        np.asarray(Vw, np.float32).T.reshape(HC, P, L).transpose(1, 0, 2))  # [P,HC,L]
    Vb2 = np.asarray(Vb, np.float32).reshape(1, L)
    in_maps = []
    for core in range(NCORES):
        cs = slice(core * CL, (core + 1) * CL)
        in_maps.append({
            "keys": np.ascontiguousarray(keys[:, :, cs, :]),
            "queriesT": np.ascontiguousarray(queries[:, 0, cs, :].transpose(0, 2, 1)),
            "KwT": KwT, "QwT": QwT, "QbR": QbR, "KbR": KbR, "VwR": VwR, "Vb": Vb2,
        })
    return in_maps


def assemble_outputs(results):
    ctx = np.concatenate([r["ctx"] for r in results], axis=0)  # [C, KL]
    wparts = []
    for r in results:
        w = r["wout"]  # [CL, P, L, TC];  w[c, p, l, tc] = weight[l, t=tc*P+p, c]
        wparts.append(np.ascontiguousarray(w.transpose(2, 3, 1, 0)).reshape(L, T, CL))
    weights = np.concatenate(wparts, axis=2).reshape(L, T, C, 1)
    return ctx.astype(np.float32), weights.astype(np.float32)


def kernel(queries, keys, Qw, Qb, Kw, Kb, Vw, Vb, _trace=False, _tmpdir=None):
    nc = build()
    in_maps = make_in_maps(queries, keys, Qw, Qb, Kw, Kb, Vw, Vb)
    res = bass_utils.run_bass_kernel_spmd(
        nc, in_maps, core_ids=list(range(NCORES)), trace=_trace, tmpdir=_tmpdir,
    )
    out = assemble_outputs(res.results)
    if _trace:
        kernel.last_results = res
    return out
